# revision 1
# baseline (speedup 1.0000x reference)
"""Trainium2 Bass kernel for additive (Bahdanau) attention.

reference:
    proj_f = features @ W1_w + W1_b          # [B, L, ATT]
    proj_h = (hidden @ W2_w + W2_b)[:, None] # [B, 1, ATT]
    scores = tanh(proj_f + proj_h) @ V_w + V_b   # [B, L]
    alpha  = softmax(scores, axis=1)
    context = einsum('bl,ble->be', alpha, features)
    returns (alpha, context)

Sharding: data-parallel over batch B=64 across 8 cores (8 examples/core).
Weights replicated. No collectives.

Per-core algorithm (X = 8 examples):
  - f32 feature loads over the 16 HWDGE queues (the only bulk HBM traffic,
    32 MB/core), DVE-convert to bf16.
  - features transposed ON-CHIP: PE transpose-mode 128x128 blocks, 4 blocks
    per PSUM bank, one strided DVE copy per bank into a per-example
    fT [128, ENCxL] tile.  (DMA-transpose via a DRAM bf16 bounce was tried
    and is queue-descriptor-bound: 71 MB through ~20 GB/s/queue.)
  - main matmul in [ATT_part, L_free] orientation: lhsT = W1 chunk
    (natural layout, bf16), rhs = fT slices.  PSUM [128, 512] f32.
  - ACT applies tanh fused with per-partition bias = (W1_b + W2_b +
    hidden @ W2_w) transposed - computed in a small prepass.
  - V-dot on PE: scores[1, 512] += V_chunk[128,1].T @ tanh_tile, accumulated
    over ATT chunks in PSUM; V-dot MMs trail the tanh by one block so the
    PE never waits on ACT.  (V_b dropped: softmax is shift-invariant.)
  - softmax per example on DVE/ACT (free-dim reduces on [1, 1024]).
  - context on DVE: scalar_tensor_tensor over fT with alpha replicated
    across partitions (gpsimd partition_broadcast).
"""

import numpy as np

B, L, ENC, DEC, ATT = 64, 1024, 1024, 1024, 1024
N_CORES = 8
X = B // N_CORES  # examples per core
P = 128
NE = ENC // P  # 8
NA = ATT // P  # 8
ND = DEC // P  # 8
LH = 512       # free-dim half for fp32 PSUM bank
NL = L // LH   # 2

_CACHE = {}


def _build():
    import concourse.bacc as bacc
    import concourse.mybir as mybir
    import concourse.tile as tile

    f32, bf16 = mybir.dt.float32, mybir.dt.bfloat16
    Tanh = mybir.ActivationFunctionType.Tanh
    Exp = mybir.ActivationFunctionType.Exp
    mult = mybir.AluOpType.mult
    AX = mybir.AxisListType.X

    nc = bacc.Bacc("TRN2", target_bir_lowering=False, debug=False, num_devices=N_CORES)

    feats = nc.declare_dram_parameter("features", [X, L, ENC], f32, isOutput=False)
    hid = nc.declare_dram_parameter("hidden_state", [X, DEC], f32, isOutput=False)
    w1 = nc.declare_dram_parameter("W1_w", [ENC, ATT], f32, isOutput=False)
    w1b = nc.declare_dram_parameter("W1_b", [ATT], f32, isOutput=False)
    w2 = nc.declare_dram_parameter("W2_w", [DEC, ATT], f32, isOutput=False)
    w2b = nc.declare_dram_parameter("W2_b", [ATT], f32, isOutput=False)
    vw = nc.declare_dram_parameter("V_w", [ATT], f32, isOutput=False)
    alpha_o = nc.declare_dram_parameter("alpha", [X, L], f32, isOutput=True)
    ctx_o = nc.declare_dram_parameter("context", [X, ENC], f32, isOutput=True)

    eye_dram = nc.inline_tensor(np.eye(P, dtype=np.float32), "eye128")

    with tile.TileContext(nc) as tc:
        with (
            tc.tile_pool(name="const", bufs=1) as const,
            tc.tile_pool(name="fn", bufs=5) as fnp,
            tc.tile_pool(name="fb", bufs=12) as fbp,
            tc.tile_pool(name="ft", bufs=3) as ftp,
            tc.tile_pool(name="mm", bufs=3, space="PSUM") as psum,
            tc.tile_pool(name="sc", bufs=3, space="PSUM") as spsum,
            tc.tile_pool(name="tp", bufs=2, space="PSUM") as tpsum,
            tc.tile_pool(name="tb", bufs=6) as tp,
            tc.tile_pool(name="jk", bufs=1) as jp,
            tc.tile_pool(name="al", bufs=2) as alp,
            tc.tile_pool(name="ms", bufs=1) as ms,
        ):
            # ---------------- prep: constants & weights ----------------
            eye = const.tile([P, P], f32, tag="eye")
            nc.sync.dma_start(eye[:], eye_dram[:, :])
            eye_bf = const.tile([P, P], bf16, tag="eye_bf")
            nc.vector.tensor_copy(eye_bf[:], eye[:])

            # small prep loads first so they never queue behind feature loads
            h_nat = ms.tile([X, DEC], f32, tag="h_nat")
            nc.sync.dma_start(h_nat[:], hid[:, :])
            b1_nat = ms.tile([1, ATT], f32, tag="b1_nat")
            nc.sync.dma_start(b1_nat[:], w1b[None, :])
            b2_nat = ms.tile([1, ATT], f32, tag="b2_nat")
            nc.sync.dma_start(b2_nat[:], w2b[None, :])
            v_nat = ms.tile([1, ATT], f32, tag="v_nat")
            nc.sync.dma_start(v_nat[:], vw[None, :])

            w1bf = []
            for e in range(NE):
                t = const.tile([P, ATT], bf16, tag=f"w1_{e}")
                nc.gpsimd.dma_start(t[:], w1[P * e : P * (e + 1), :])
                w1bf.append(t)

            w2t = []
            for e in range(ND):
                t = const.tile([P, ATT], bf16, tag=f"w2_{e}")
                nc.gpsimd.dma_start(t[:], w2[P * e : P * (e + 1), :])
                w2t.append(t)

            # ---------------- per-example staging ----------------
            def emit_load_convert(x):
                # f32 loads split 4-ways per l-chunk to spread queues,
                # then DVE bf16 convert.  The right half (e-chunks 4..7) is
                # stored to a DRAM scratch for the DMA-transpose path.
                fbs = []
                for c in range(8):
                    fnat = fnp.tile([P, ENC], f32, tag="fn")
                    fb = fbp.tile([P, ENC], bf16, tag="fb")
                    for hh in range(2):
                        for q in range(2 * hh, 2 * hh + 2):
                            nc.sync.dma_start(
                                fnat[:, 256 * q : 256 * (q + 1)],
                                feats[x, P * c : P * (c + 1), 256 * q : 256 * (q + 1)],
                            )
                        nc.vector.tensor_copy(
                            fb[:, LH * hh : LH * (hh + 1)], fnat[:, LH * hh : LH * (hh + 1)]
                        )
                    fbs.append(fb)
                return fbs

            def alloc_ft():
                return ms_ft()

            def ms_ft():
                t = ftp.tile([P, NE * L], bf16, tag="ft")
                return t

            def ft_view(ft):
                return ft.rearrange("p (e lc c) -> p e lc c", e=NE, lc=NE)

            def emit_transpose_group(fbs, ft, lc, h):
                # transpose blocks (e in [4h, 4h+4), l-chunk lc) of the
                # natural bf16 tiles into ft columns e*L + 128*lc.
                tps = tpsum.tile([P, 4 * P], bf16, tag="tp")
                for j in range(4):
                    e = 4 * h + j
                    nc.tensor.transpose(
                        tps[:, P * j : P * (j + 1)],
                        fbs[lc][:, P * e : P * (e + 1)],
                        eye_bf[:],
                    )
                dst = ft_view(ft)[:, 4 * h : 4 * h + 4, lc, :]
                src = tps.rearrange("p (e c) -> p e c", e=4)
                nc.vector.tensor_copy(dst, src)

            # ---------------- main per-example pipeline ----------------
            # V-dot matmuls trail the tanh by one block so the PE never
            # waits on ACT.
            pending = []

            def flush_pending():
                for sc_ap, vw_ap, tb_ap, st, sp in pending:
                    nc.tensor.matmul(sc_ap, vw_ap, tb_ap, start=st, stop=sp)
                pending.clear()

            # prologue: stage examples 0 and 1
            fb_map = {}
            fb_map[0] = emit_load_convert(0)
            ft_map = {0: alloc_ft()}
            # only the lh=0 half of example 0's transposes before the body;
            # the rest interleave with the first main-MM blocks.
            for lc in range(4):
                for h in range(2):
                    emit_transpose_group(fb_map[0], ft_map[0], lc, h)
            fb_map[1] = emit_load_convert(1)

            # hT_all[p, c, x] = hid[x, 128c + p] via natural load + PE transpose
            hn_bf = ms.tile([X, DEC], bf16, tag="hn_bf")
            nc.vector.tensor_copy(hn_bf[:], h_nat[:])
            hTb = ms.tile([P, ND, X], bf16, tag="hTb")
            for c in range(ND):
                tps_h = tpsum.tile([P, X], bf16, tag="tp", name=f"tpsh{c}")
                nc.tensor.transpose(tps_h[:], hn_bf[:, P * c : P * (c + 1)], eye_bf[0:X, 0:X])
                nc.vector.tensor_copy(hTb[:, c, :], tps_h[:])

            # bias vectors: natural load, PE-transpose each into [128, NA],
            # then add the two bias transposes (both at partition 0).
            def load_transposed_vec(nat, name, dt):
                tps_v = tpsum.tile([P, NA], f32, tag="tp", name=f"tps_{name}")
                for c in range(NA):
                    nc.tensor.transpose(
                        tps_v[:, c : c + 1], nat[:, P * c : P * (c + 1)], eye[0:1, 0:1]
                    )
                dst = ms.tile([P, NA], dt, tag=name, name=name)
                nc.vector.tensor_copy(dst[:], tps_v[:])
                return dst

            b1T = load_transposed_vec(b1_nat, "b1T", f32)
            b2T = load_transposed_vec(b2_nat, "b2T", f32)
            vwbf = load_transposed_vec(v_nat, "vwbf", bf16)
            bT = ms.tile([P, NA], f32, tag="bT")
            nc.vector.tensor_add(bT[:], b1T[:], b2T[:])

            # proj_h transposed, plus bias: phb[p, a, x]
            phb = ms.tile([P, NA, X], f32, tag="phb")
            for a in range(NA):
                ph_ps = psum.tile([P, X], f32, tag="mm")
                for e in range(ND):
                    nc.tensor.matmul(
                        ph_ps[:],
                        w2t[e][:, P * a : P * (a + 1)],
                        hTb[:, e, :],
                        start=(e == 0),
                        stop=(e == ND - 1),
                    )
                nc.vector.tensor_scalar_add(phb[:, a, :], ph_ps[:], bT[:, a : a + 1])



            for x in range(X):
                if x + 2 < X:
                    fb_map[x + 2] = emit_load_convert(x + 2)
                if x + 1 < X:
                    ft_map[x + 1] = alloc_ft()
                    tgroups = [(x + 1, lc, h) for lc in range(NE) for h in range(2)]
                else:
                    tgroups = []
                if x == 0:
                    tgroups = [(0, lc, h) for lc in range(4, NE) for h in range(2)] + tgroups
                    blocks = [(a, 0) for a in range(NA)] + [(a, 1) for a in range(NA)]
                    rate, mb0 = 2, 1
                else:
                    blocks = [(a, lh) for a in range(NA) for lh in range(NL)]
                    rate, mb0 = 1, 2
                ft = ft_map[x]
                ftv = ft_view(ft)

                sc_h = {}
                mb = 0  # micro-block index
                for a, lh in blocks:
                    pp = psum.tile([P, LH], f32, tag="mm")
                    for e in range(NE):
                        nc.tensor.matmul(
                            pp[:],
                            w1bf[e][:, P * a : P * (a + 1)],
                            ft[:, e * L + LH * lh : e * L + LH * (lh + 1)],
                            start=(e == 0),
                            stop=(e == NE - 1),
                        )
                        if e == 1:
                            flush_pending()
                    # stage upcoming transposes between MM blocks
                    if mb >= mb0:
                        for _ in range(rate):
                            if tgroups:
                                xi, lc, h = tgroups.pop(0)
                                emit_transpose_group(fb_map[xi], ft_map[xi], lc, h)
                    mb += 1
                    if lh not in sc_h:
                        sc_h[lh] = spsum.tile([1, LH], f32, tag="sc", name=f"sch{lh}")
                    tb = tp.tile([P, LH], bf16, tag="tb")
                    nc.scalar.activation(tb[:], pp[:], Tanh, bias=phb[:, a, x : x + 1])
                    pending.append(
                        (sc_h[lh][:], vwbf[:, a : a + 1], tb[:], a == 0, a == NA - 1)
                    )

                flush_pending()
                for xi, lc, h in tgroups:
                    emit_transpose_group(fb_map[xi], ft_map[xi], lc, h)
                if x + 1 < X:
                    fb_map.pop(x, None)

                # softmax over L on partition 0
                scores = alp.tile([1, L], f32, tag="scores")
                nc.vector.tensor_copy(scores[:, 0:LH], sc_h[0][:])
                nc.vector.tensor_copy(scores[:, LH:L], sc_h[1][:])
                negm = alp.tile([1, 1], f32, tag="negm")
                nc.vector.tensor_reduce(
                    negm[:], scores[:], axis=AX, op=mybir.AluOpType.max, negate=True
                )
                esb = alp.tile([1, L], f32, tag="esb")
                ssum = alp.tile([1, 1], f32, tag="ssum")
                nc.scalar.activation(esb[:], scores[:], Exp, bias=negm[:], accum_out=ssum[:])
                rinv = alp.tile([1, 1], f32, tag="rinv")
                nc.vector.reciprocal(rinv[:], ssum[:])
                a32 = alp.tile([1, L], f32, tag="a32")
                nc.vector.tensor_scalar_mul(a32[:], esb[:], rinv[:])
                nc.sync.dma_start(alpha_o[x, :], a32[:])
                if x < X - 1:
                    abf = alp.tile([1, L], bf16, tag="abf")
                    nc.vector.tensor_scalar_mul(abf[:], esb[:], rinv[:])
                    arep = alp.tile([P, L], bf16, tag="arep")
                    nc.gpsimd.partition_broadcast(arep[:], abf[:])

                    # context: ctx[e-chunk] = sum_l fT[e][:, l] * alpha[l]
                    ctx_x = alp.tile([P, NE], f32, tag="ctx_x")
                    for e in range(NE):
                        jk = jp.tile([P, L], f32, tag="jk")
                        nc.vector.scalar_tensor_tensor(
                            out=jk[:],
                            in0=ft[:, e * L : (e + 1) * L],
                            scalar=1.0,
                            in1=arep[:],
                            op0=mult,
                            op1=mult,
                            accum_out=ctx_x[:, e : e + 1],
                        )
                    # transpose [128, 8] -> [8, 128] and ship context[x] out
                    ct_ps = tpsum.tile([X, P], f32, tag="tp", name=f"ctps{x}")
                    nc.tensor.transpose(ct_ps[:], ctx_x[:], eye[:])
                    ctr = alp.tile([X, P], f32, tag="ctr")
                    nc.vector.tensor_copy(ctr[:], ct_ps[:])
                    nc.sync.dma_start(ctx_o.rearrange("x (e c) -> x e c", e=NE)[x], ctr[:])
                else:
                    # last example: context on PE (shorter serial tail than the
                    # DVE path): alphaT via PE transpose, then 16 accumulating
                    # matmuls against the natural bf16 feature tiles.
                    tps_a = tpsum.tile([P, NE], f32, tag="tp", name="tpsa")
                    for lc in range(NE):
                        nc.tensor.transpose(
                            tps_a[:, lc : lc + 1], a32[:, P * lc : P * (lc + 1)], eye[0:1, 0:1]
                        )
                    alT = alp.tile([P, NE], bf16, tag="alT")
                    nc.vector.tensor_copy(alT[:], tps_a[:])
                    fb_last = fb_map[x]
                    ctr2 = alp.tile([1, ENC], f32, tag="ctr2")
                    for eh in range(2):
                        cps = spsum.tile([1, LH], f32, tag="sc", name=f"cps{eh}")
                        for lc in range(NE):
                            nc.tensor.matmul(
                                cps[:],
                                alT[:, lc : lc + 1],
                                fb_last[lc][:, LH * eh : LH * (eh + 1)],
                                start=(lc == 0),
                                stop=(lc == NE - 1),
                            )
                        nc.vector.tensor_copy(ctr2[:, LH * eh : LH * (eh + 1)], cps[:])
                    nc.sync.dma_start(ctx_o[x, :], ctr2[:])


    nc.compile()
    return nc


def kernel(features, hidden_state, W1_w, W1_b, W2_w, W2_b, V_w, V_b):
    from concourse.bass_utils import run_bass_kernel_spmd

    if "nc" not in _CACHE:
        _CACHE["nc"] = _build()
    nc = _CACHE["nc"]

    features = np.ascontiguousarray(np.asarray(features, dtype=np.float32))
    hidden_state = np.ascontiguousarray(np.asarray(hidden_state, dtype=np.float32))
    W1_w = np.ascontiguousarray(np.asarray(W1_w, dtype=np.float32))
    W1_b = np.ascontiguousarray(np.asarray(W1_b, dtype=np.float32))
    W2_w = np.ascontiguousarray(np.asarray(W2_w, dtype=np.float32))
    W2_b = np.ascontiguousarray(np.asarray(W2_b, dtype=np.float32))
    V_w = np.ascontiguousarray(np.asarray(V_w, dtype=np.float32))

    in_maps = []
    for c in range(N_CORES):
        in_maps.append(
            {
                "features": np.ascontiguousarray(features[c * X : (c + 1) * X]),
                "hidden_state": np.ascontiguousarray(hidden_state[c * X : (c + 1) * X]),
                "W1_w": W1_w,
                "W1_b": W1_b,
                "W2_w": W2_w,
                "W2_b": W2_b,
                "V_w": V_w,
            }
        )

    res = run_bass_kernel_spmd(nc, in_maps, list(range(N_CORES)), **_CACHE.get("run_kwargs", {}))
    _CACHE["last_result"] = res
    alpha = np.concatenate([res.results[c]["alpha"] for c in range(N_CORES)], axis=0)
    context = np.concatenate([res.results[c]["context"] for c in range(N_CORES)], axis=0)
    return alpha, context



# revision 4
# speedup vs baseline: 1.0257x; 1.0257x over previous
"""Trainium2 Bass kernel for additive (Bahdanau) attention — fp8 DoubleRow.

reference:
    proj_f = features @ W1_w + W1_b          # [B, L, ATT]
    proj_h = (hidden @ W2_w + W2_b)[:, None] # [B, 1, ATT]
    scores = tanh(proj_f + proj_h) @ V_w + V_b   # [B, L]
    alpha  = softmax(scores, axis=1)
    context = einsum('bl,ble->be', alpha, features)
    returns (alpha, context)

Sharding: data-parallel over batch B=64 across 8 cores (8 examples/core).
Weights replicated. No collectives.

Per-core algorithm (X = 8 examples), evolved from the bf16 baseline:
  - main GEMM and V-dot run in fp8e4 with perf_mode=DoubleRow (2 fp8
    weights/PE cell, K=256 per matmul).  To dodge e4m3 subnormals,
    features are scaled x32 and W1/V x64; the tanh activation descales
    via its scale arg (2^-11) and exp via scale 1/64 (softmax is
    shift-invariant so V_b is dropped, and the max-subtract works on the
    x64 scores with a rescaled bias).
  - context needs bf16 features for accuracy (fp8 features give ~3e-2
    rel err), so TWO transposed copies are kept: ft16 (bf16, feeds the
    DVE context STT) and ft8 = fp8(32 * ft16) via one strided DVE cast.
  - f32->bf16 feature casts moved from DVE to the Scalar engine (ACT),
    doled out two per GEMM block so they never head-of-line-block the
    tanh ops behind slow DMA arrivals.
  - software pipeline per example x's 16 GEMM blocks:
      blocks 0-3   : (DMA for x+2 chunks 0-3 issued at loop top)
      blocks 4-7   : ACT casts for x+2 chunks 0-3
      block  8     : DMA issue for x+2 chunks 4-7
      blocks 8-15  : PE transposes for x+1 (2 groups/block) + ft8 casts
      blocks 12-15 : ACT casts for x+2 chunks 4-7
  - scores accumulate in PSUM [1, 512] via DoubleRow V-dot matmuls that
    trail the tanh by one block so the PE never waits on ACT.
  - context on DVE: scalar_tensor_tensor over ft16 with alpha replicated
    across partitions (gpsimd partition_broadcast); bf16 product scratch.
    Last example: context on PE against the natural bf16 tiles (shorter
    serial tail).
"""

import numpy as np

B, L, ENC, DEC, ATT = 64, 1024, 1024, 1024, 1024
N_CORES = 8
X = B // N_CORES  # examples per core
P = 128
NE = ENC // P  # 8
NA = ATT // P  # 8
ND = DEC // P  # 8
LH = 512       # free-dim half for fp32 PSUM bank
NL = L // LH   # 2

FSCALE = 32.0                      # feature scale into fp8
WSCALE = 64.0                      # W1 / V scale into fp8
PSCALE = 1.0 / (FSCALE * WSCALE)   # descale inside tanh
SSCALE = 1.0 / WSCALE              # descale inside exp

_CACHE = {}


def _build():
    import concourse.bacc as bacc
    import concourse.mybir as mybir
    import concourse.tile as tile

    f32, bf16 = mybir.dt.float32, mybir.dt.bfloat16
    fp8 = mybir.dt.float8e4
    Tanh = mybir.ActivationFunctionType.Tanh
    Exp = mybir.ActivationFunctionType.Exp
    DR = mybir.MatmulPerfMode.DoubleRow
    mult = mybir.AluOpType.mult
    AX = mybir.AxisListType.X

    nc = bacc.Bacc("TRN2", target_bir_lowering=False, debug=False, num_devices=N_CORES)

    feats = nc.declare_dram_parameter("features", [X, L, ENC], f32, isOutput=False)
    hid = nc.declare_dram_parameter("hidden_state", [X, DEC], f32, isOutput=False)
    w1 = nc.declare_dram_parameter("W1_w", [ENC, ATT], f32, isOutput=False)
    w1b = nc.declare_dram_parameter("W1_b", [ATT], f32, isOutput=False)
    w2 = nc.declare_dram_parameter("W2_w", [DEC, ATT], f32, isOutput=False)
    w2b = nc.declare_dram_parameter("W2_b", [ATT], f32, isOutput=False)
    vw = nc.declare_dram_parameter("V_w", [ATT], f32, isOutput=False)
    alpha_o = nc.declare_dram_parameter("alpha", [X, L], f32, isOutput=True)
    ctx_o = nc.declare_dram_parameter("context", [X, ENC], f32, isOutput=True)

    eye_dram = nc.inline_tensor(np.eye(P, dtype=np.float32), "eye128")

    with tile.TileContext(nc) as tc:
        with (
            tc.tile_pool(name="const", bufs=1) as const,
            tc.tile_pool(name="fn", bufs=6) as fnp,
            tc.tile_pool(name="fb", bufs=16) as fbp,
            tc.tile_pool(name="ft", bufs=3) as ftp,
            tc.tile_pool(name="f8", bufs=2) as f8p,
            tc.tile_pool(name="mm", bufs=3, space="PSUM") as psum,
            tc.tile_pool(name="sc", bufs=3, space="PSUM") as spsum,
            tc.tile_pool(name="tp", bufs=2, space="PSUM") as tpsum,
            tc.tile_pool(name="tb", bufs=6) as tp,
            tc.tile_pool(name="jk", bufs=1) as jp,
            tc.tile_pool(name="al", bufs=2) as alp,
            tc.tile_pool(name="ms", bufs=1) as ms,
        ):
            # ---------------- prep: constants & weights ----------------
            eye = const.tile([P, P], f32, tag="eye")
            nc.sync.dma_start(eye[:], eye_dram[:, :])
            eye_bf = const.tile([P, P], bf16, tag="eye_bf")
            nc.vector.tensor_copy(eye_bf[:], eye[:])

            # small prep loads first so they never queue behind feature loads
            h_nat = alp.tile([X, DEC], f32, tag="esb", name="h_nat")
            nc.sync.dma_start(h_nat[:], hid[:, :])
            b1_nat = alp.tile([1, ATT], f32, tag="scores", name="b1_nat")
            nc.sync.dma_start(b1_nat[:], w1b[None, :])
            b2_nat = alp.tile([1, ATT], f32, tag="esb", name="b2_nat")
            nc.sync.dma_start(b2_nat[:], w2b[None, :])
            v_nat = alp.tile([1, ATT], f32, tag="scores", name="v_nat")
            nc.sync.dma_start(v_nat[:], vw[None, :])

            # W1 -> fp8 x64 in DoubleRow pair layout: w1d[q][p, i, m] =
            # 64*W1[(2q+i)*128 + p, m]
            w1d = []
            for q in range(NE // 2):
                t = const.tile([P, 2, ATT], fp8, tag=f"w1d_{q}")
                w1d.append(t)
            for e in range(NE):
                stage = fnp.tile([P, ATT], f32, tag="fn", name=f"w1s{e}")
                nc.sync.dma_start(stage[:], w1[P * e : P * (e + 1), :])
                q, i = divmod(e, 2)
                nc.vector.tensor_scalar_mul(w1d[q][:, i, :], stage[:], WSCALE)

            w2t = []
            for e in range(ND):
                t = const.tile([P, ATT], bf16, tag=f"w2_{e}")
                nc.gpsimd.dma_start(t[:], w2[P * e : P * (e + 1), :])
                w2t.append(t)

            # ---------------- per-example staging helpers ----------------
            fnat_map = {}   # (x, c) -> f32 natural tile
            fb_map = {}     # x -> {c: bf16 natural tile}
            ft_map = {}     # x -> ft16 tile [P, NE*L] bf16
            f8_map = {}     # x -> ft8 tile  [P, NE*L] fp8

            def emit_dma_half(x, half):
                for c in range(4 * half, 4 * half + 4):
                    fnat = fnp.tile([P, ENC], f32, tag="fn", name=f"fn{x}_{c}")
                    fnat_map[(x, c)] = fnat
                    for q2 in range(4):
                        nc.sync.dma_start(
                            fnat[:, 256 * q2 : 256 * (q2 + 1)],
                            feats[x, P * c : P * (c + 1), 256 * q2 : 256 * (q2 + 1)],
                        )

            def emit_cast(x, k):
                # k = 2*c + hh : cast half hh of chunk c, f32 -> bf16 on ACT
                c, hh = divmod(k, 2)
                if hh == 0:
                    fb_map.setdefault(x, {})[c] = fbp.tile([P, ENC], bf16, tag="fb", name=f"fb{x}_{c}")
                fb = fb_map[x][c]
                nc.scalar.copy(
                    fb[:, LH * hh : LH * (hh + 1)],
                    fnat_map[(x, c)][:, LH * hh : LH * (hh + 1)],
                )
                if hh == 1:
                    fnat_map.pop((x, c))

            def ft_view(ft):
                return ft.rearrange("p (e lc c) -> p e lc c", e=NE, lc=NE)

            def emit_transpose_group(x, g):
                # g = 2*lc + h : transpose blocks (e in [4h, 4h+4), l-chunk lc)
                lc, h = divmod(g, 2)
                if g == 0:
                    ft_map[x] = ftp.tile([P, NE * L], bf16, tag="ft", name=f"ft{x}")
                ft = ft_map[x]
                fb = fb_map[x][lc]
                tps = tpsum.tile([P, 4 * P], bf16, tag="tp")
                for j in range(4):
                    e = 4 * h + j
                    nc.tensor.transpose(
                        tps[:, P * j : P * (j + 1)],
                        fb[:, P * e : P * (e + 1)],
                        eye_bf[:],
                    )
                dst = ft_view(ft)[:, 4 * h : 4 * h + 4, lc, :]
                src = tps.rearrange("p (e c) -> p e c", e=4)
                nc.vector.tensor_copy(dst, src)

            def emit_ft8(x, half):
                # scaled fp8 copy of the l-range [512*half, 512*(half+1))
                if half == 0:
                    f8_map[x] = f8p.tile([P, NE * L], fp8, tag="f8", name=f"f8{x}")
                src = ft_view(ft_map[x])[:, :, 4 * half : 4 * half + 4, :]
                dst = ft_view(f8_map[x])[:, :, 4 * half : 4 * half + 4, :]
                nc.vector.tensor_scalar_mul(dst, src, FSCALE)

            # ---------------- prologue ----------------
            emit_dma_half(0, 0)
            emit_dma_half(0, 1)
            for k in range(16):
                emit_cast(0, k)
            emit_dma_half(1, 0)
            emit_dma_half(1, 1)

            # hT_all[p, c, x] = hid[x, 128c + p] via natural load + PE transpose
            hn_bf = ms.tile([X, DEC], bf16, tag="hn_bf")
            nc.vector.tensor_copy(hn_bf[:], h_nat[:])
            hTb = ms.tile([P, ND, X], bf16, tag="hTb")
            for c in range(ND):
                tps_h = tpsum.tile([P, X], bf16, tag="tp", name=f"tpsh{c}")
                nc.tensor.transpose(tps_h[:], hn_bf[:, P * c : P * (c + 1)], eye_bf[0:X, 0:X])
                nc.vector.tensor_copy(hTb[:, c, :], tps_h[:])

            # bias / V vectors transposed into [128, NA] (partition = within-chunk)
            def load_transposed_vec(nat, name):
                tps_v = tpsum.tile([P, NA], f32, tag="tp", name=f"tps_{name}")
                for c in range(NA):
                    nc.tensor.transpose(
                        tps_v[:, c : c + 1], nat[:, P * c : P * (c + 1)], eye[0:1, 0:1]
                    )
                dst = ms.tile([P, NA], f32, tag=name, name=name)
                nc.vector.tensor_copy(dst[:], tps_v[:])
                return dst

            b1T = load_transposed_vec(b1_nat, "b1T")
            b2T = load_transposed_vec(b2_nat, "b2T")
            vT = load_transposed_vec(v_nat, "vT")
            bT = ms.tile([P, NA], f32, tag="bT")
            nc.vector.tensor_add(bT[:], b1T[:], b2T[:])

            # V in fp8 x64 DoubleRow pair layout: vwd[p, i, j] = 64*V[(2j+i)*128+p]
            # (free-dim padded to 16 so the Ko step is 16 B)
            vwd = ms.tile([P, 2, 16], fp8, tag="vwd")
            nc.vector.tensor_scalar_mul(
                vwd[:, :, 0:4], vT.rearrange("p (j two) -> p two j", two=2), WSCALE
            )

            # proj_h transposed, plus bias: phb[p, a, x]
            phb = ms.tile([P, NA, X], f32, tag="phb")
            for a in range(NA):
                ph_ps = psum.tile([P, X], f32, tag="mm")
                for e in range(ND):
                    nc.tensor.matmul(
                        ph_ps[:],
                        w2t[e][:, P * a : P * (a + 1)],
                        hTb[:, e, :],
                        start=(e == 0),
                        stop=(e == ND - 1),
                    )
                nc.vector.tensor_scalar_add(phb[:, a, :], ph_ps[:], bT[:, a : a + 1])

            # stage example 0 fully (transposes + fp8 cast), example 1 casts
            for g in range(16):
                emit_transpose_group(0, g)
                if g == 7:
                    emit_ft8(0, 0)
            emit_ft8(0, 1)
            for k in range(16):
                emit_cast(1, k)

            # ---------------- main per-example pipeline ----------------
            pending = []

            def flush_pending():
                for sc_ap, j, tb_ap in pending:
                    nc.tensor.matmul(
                        sc_ap,
                        vwd[:, :, j : j + 1],
                        tb_ap,
                        start=(j == 0),
                        stop=(j == 3),
                        perf_mode=DR,
                    )
                pending.clear()

            for x in range(X):
                cast_for = x + 2 if x + 2 < X else None
                trans_for = x + 1 if x + 1 < X else None
                if cast_for is not None:
                    emit_dma_half(cast_for, 0)

                ft8v = f8_map[x].rearrange("p (e l) -> p e l", e=NE)
                sc_h = {}
                tb3 = None
                for b in range(16):
                    lh, a = divmod(b, 8)
                    j, i = divmod(a, 2)
                    if a == 0:
                        sc_h[lh] = spsum.tile([1, LH], f32, tag="sc", name=f"sch{x}_{lh}")
                    if i == 0:
                        tb3 = tp.tile([P, 2, LH], fp8, tag="tb")
                    pp = psum.tile([P, LH], f32, tag="mm")
                    for q in range(4):
                        nc.tensor.matmul(
                            pp[:],
                            w1d[q][:, :, P * a : P * (a + 1)],
                            ft8v[:, 2 * q : 2 * q + 2, LH * lh : LH * (lh + 1)],
                            start=(q == 0),
                            stop=(q == 3),
                            perf_mode=DR,
                        )
                        if q == 1:
                            flush_pending()

                    # scheduled pipeline work for later examples
                    if cast_for is not None:
                        if b == 8:
                            emit_dma_half(cast_for, 1)
                        if 4 <= b < 8:
                            emit_cast(cast_for, 2 * (b - 4))
                            emit_cast(cast_for, 2 * (b - 4) + 1)
                        elif 12 <= b:
                            emit_cast(cast_for, 2 * (b - 12) + 8)
                            emit_cast(cast_for, 2 * (b - 12) + 9)
                    if trans_for is not None and b >= 8:
                        emit_transpose_group(trans_for, 2 * (b - 8))
                        emit_transpose_group(trans_for, 2 * (b - 8) + 1)
                        if b == 11:
                            emit_ft8(trans_for, 0)
                        elif b == 15:
                            emit_ft8(trans_for, 1)

                    nc.scalar.activation(
                        tb3[:, i, :], pp[:], Tanh,
                        bias=phb[:, a, x : x + 1], scale=PSCALE,
                    )
                    if i == 1:
                        pending.append((sc_h[lh][:], j, tb3[:]))

                flush_pending()

                # softmax over L on partition 0 (scores carry a x64 scale)
                scores = alp.tile([1, L], f32, tag="scores")
                nc.vector.tensor_copy(scores[:, 0:LH], sc_h[0][:])
                nc.vector.tensor_copy(scores[:, LH:L], sc_h[1][:])
                negm = alp.tile([1, 1], f32, tag="negm")
                nc.vector.tensor_reduce(
                    negm[:], scores[:], axis=AX, op=mybir.AluOpType.max, negate=True
                )
                negm_s = alp.tile([1, 1], f32, tag="negm_s")
                nc.vector.tensor_scalar_mul(negm_s[:], negm[:], SSCALE)
                esb = alp.tile([1, L], f32, tag="esb")
                ssum = alp.tile([1, 1], f32, tag="ssum")
                nc.scalar.activation(
                    esb[:], scores[:], Exp, bias=negm_s[:], scale=SSCALE,
                    accum_out=ssum[:],
                )
                rinv = alp.tile([1, 1], f32, tag="rinv")
                nc.vector.reciprocal(rinv[:], ssum[:])
                a32 = alp.tile([1, L], f32, tag="scores", name=f"a32_{x}")
                nc.vector.tensor_scalar_mul(a32[:], esb[:], rinv[:])
                nc.sync.dma_start(alpha_o[x, :], a32[:])

                if x < X - 1:
                    abf = alp.tile([1, L], bf16, tag="abf")
                    nc.vector.tensor_scalar_mul(abf[:], esb[:], rinv[:])
                    arep = alp.tile([P, L], bf16, tag="arep")
                    nc.gpsimd.partition_broadcast(arep[:], abf[:])

                    # context: ctx[e-chunk] = sum_l fT[e][:, l] * alpha[l]
                    ft = ft_map[x]
                    ctx_x = alp.tile([P, NE], f32, tag="ctx_x")
                    for e in range(NE):
                        jk = jp.tile([P, L], bf16, tag="jk")
                        nc.vector.scalar_tensor_tensor(
                            out=jk[:],
                            in0=ft[:, e * L : (e + 1) * L],
                            scalar=1.0,
                            in1=arep[:],
                            op0=mult,
                            op1=mult,
                            accum_out=ctx_x[:, e : e + 1],
                        )
                    ct_ps = tpsum.tile([X, P], f32, tag="tp", name=f"ctps{x}")
                    nc.tensor.transpose(ct_ps[:], ctx_x[:], eye[:])
                    ctr = alp.tile([X, P], f32, tag="ctr")
                    nc.vector.tensor_copy(ctr[:], ct_ps[:])
                    nc.sync.dma_start(ctx_o.rearrange("x (e c) -> x e c", e=NE)[x], ctr[:])
                else:
                    # last example: context on PE against natural bf16 tiles
                    tps_a = tpsum.tile([P, NE], f32, tag="tp", name="tpsa")
                    for lc in range(NE):
                        nc.tensor.transpose(
                            tps_a[:, lc : lc + 1], a32[:, P * lc : P * (lc + 1)], eye[0:1, 0:1]
                        )
                    alT = alp.tile([P, NE], bf16, tag="alT")
                    nc.vector.tensor_copy(alT[:], tps_a[:])
                    fb_last = fb_map[x]
                    ctr2 = alp.tile([1, ENC], f32, tag="esb", name="ctr2")
                    for eh in range(2):
                        cps = spsum.tile([1, LH], f32, tag="sc", name=f"cps{eh}")
                        for lc in range(NE):
                            nc.tensor.matmul(
                                cps[:],
                                alT[:, lc : lc + 1],
                                fb_last[lc][:, LH * eh : LH * (eh + 1)],
                                start=(lc == 0),
                                stop=(lc == NE - 1),
                            )
                        nc.vector.tensor_copy(ctr2[:, LH * eh : LH * (eh + 1)], cps[:])
                    nc.sync.dma_start(ctx_o[x, :], ctr2[:])

    nc.compile()
    return nc


def kernel(features, hidden_state, W1_w, W1_b, W2_w, W2_b, V_w, V_b):
    from concourse.bass_utils import run_bass_kernel_spmd

    if "nc" not in _CACHE:
        _CACHE["nc"] = _build()
    nc = _CACHE["nc"]

    features = np.ascontiguousarray(np.asarray(features, dtype=np.float32))
    hidden_state = np.ascontiguousarray(np.asarray(hidden_state, dtype=np.float32))
    W1_w = np.ascontiguousarray(np.asarray(W1_w, dtype=np.float32))
    W1_b = np.ascontiguousarray(np.asarray(W1_b, dtype=np.float32))
    W2_w = np.ascontiguousarray(np.asarray(W2_w, dtype=np.float32))
    W2_b = np.ascontiguousarray(np.asarray(W2_b, dtype=np.float32))
    V_w = np.ascontiguousarray(np.asarray(V_w, dtype=np.float32))

    in_maps = []
    for c in range(N_CORES):
        in_maps.append(
            {
                "features": np.ascontiguousarray(features[c * X : (c + 1) * X]),
                "hidden_state": np.ascontiguousarray(hidden_state[c * X : (c + 1) * X]),
                "W1_w": W1_w,
                "W1_b": W1_b,
                "W2_w": W2_w,
                "W2_b": W2_b,
                "V_w": V_w,
            }
        )

    res = run_bass_kernel_spmd(nc, in_maps, list(range(N_CORES)), **_CACHE.get("run_kwargs", {}))
    _CACHE["last_result"] = res
    alpha = np.concatenate([res.results[c]["alpha"] for c in range(N_CORES)], axis=0)
    context = np.concatenate([res.results[c]["context"] for c in range(N_CORES)], axis=0)
    return alpha, context


# revision 9
# speedup vs baseline: 1.1111x; 1.0832x over previous
"""Trainium2 Bass kernel for additive (Bahdanau) attention — fp8 DoubleRow.

reference:
    proj_f = features @ W1_w + W1_b          # [B, L, ATT]
    proj_h = (hidden @ W2_w + W2_b)[:, None] # [B, 1, ATT]
    scores = tanh(proj_f + proj_h) @ V_w + V_b   # [B, L]
    alpha  = softmax(scores, axis=1)
    context = einsum('bl,ble->be', alpha, features)
    returns (alpha, context)

Sharding: data-parallel over batch B=64 across 8 cores (8 examples/core).
Weights replicated. No collectives.

Per-core algorithm (X = 8 examples), evolved from the bf16 baseline:
  - main GEMM and V-dot run in fp8e4 with perf_mode=DoubleRow (2 fp8
    weights/PE cell, K=256 per matmul).  To dodge e4m3 subnormals,
    features are scaled x32 and W1/V x64; the tanh activation descales
    via its scale arg (2^-11) and exp via scale 1/64 (softmax is
    shift-invariant so V_b is dropped, and the max-subtract works on the
    x64 scores with a rescaled bias).
  - context needs bf16 features for accuracy (fp8 features give ~3e-2
    rel err), so TWO transposed copies are kept: ft16 (bf16, feeds the
    DVE context STT) and ft8 = fp8(32 * ft16) via one strided DVE cast.
  - f32->bf16 feature casts moved from DVE to the Scalar engine (ACT),
    doled out two per GEMM block so they never head-of-line-block the
    tanh ops behind slow DMA arrivals.
  - software pipeline per example x's 16 GEMM blocks:
      blocks 0-3   : (DMA for x+2 chunks 0-3 issued at loop top)
      blocks 4-7   : ACT casts for x+2 chunks 0-3
      block  8     : DMA issue for x+2 chunks 4-7
      blocks 8-15  : PE transposes for x+1 (2 groups/block) + ft8 casts
      blocks 12-15 : ACT casts for x+2 chunks 4-7
  - scores accumulate in PSUM [1, 512] via DoubleRow V-dot matmuls that
    trail the tanh by one block so the PE never waits on ACT.
  - context on DVE: scalar_tensor_tensor over ft16 with alpha replicated
    across partitions (gpsimd partition_broadcast); bf16 product scratch.
    Last example: context on PE against the natural bf16 tiles (shorter
    serial tail).
"""

import numpy as np

B, L, ENC, DEC, ATT = 64, 1024, 1024, 1024, 1024
N_CORES = 8
X = B // N_CORES  # examples per core
P = 128
NE = ENC // P  # 8
NA = ATT // P  # 8
ND = DEC // P  # 8
LH = 512       # free-dim half for fp32 PSUM bank
NL = L // LH   # 2

FSCALE = 32.0                      # feature scale into fp8
WSCALE = 64.0                      # W1 / V scale into fp8
PSCALE = 1.0 / (FSCALE * WSCALE)   # descale inside tanh
SSCALE = 1.0 / WSCALE              # descale inside exp

_CACHE = {}


def _build():
    import concourse.bacc as bacc
    import concourse.mybir as mybir
    import concourse.tile as tile

    f32, bf16 = mybir.dt.float32, mybir.dt.bfloat16
    fp8 = mybir.dt.float8e4
    Tanh = mybir.ActivationFunctionType.Tanh
    Exp = mybir.ActivationFunctionType.Exp
    DR = mybir.MatmulPerfMode.DoubleRow
    mult = mybir.AluOpType.mult
    AX = mybir.AxisListType.X

    nc = bacc.Bacc("TRN2", target_bir_lowering=False, debug=False, num_devices=N_CORES)

    feats = nc.declare_dram_parameter("features", [X, L, ENC], f32, isOutput=False)
    hid = nc.declare_dram_parameter("hidden_state", [X, DEC], f32, isOutput=False)
    w1 = nc.declare_dram_parameter("W1_w", [ENC, ATT], f32, isOutput=False)
    w1b = nc.declare_dram_parameter("W1_b", [ATT], f32, isOutput=False)
    w2 = nc.declare_dram_parameter("W2_w", [DEC, ATT], f32, isOutput=False)
    w2b = nc.declare_dram_parameter("W2_b", [ATT], f32, isOutput=False)
    vw = nc.declare_dram_parameter("V_w", [ATT], f32, isOutput=False)
    alpha_o = nc.declare_dram_parameter("alpha", [X, L], f32, isOutput=True)
    ctx_o = nc.declare_dram_parameter("context", [X, ENC], f32, isOutput=True)

    eye_dram = nc.inline_tensor(np.eye(P, dtype=np.float32), "eye128")

    with tile.TileContext(nc) as tc:
        with (
            tc.tile_pool(name="const", bufs=1) as const,
            tc.tile_pool(name="fn", bufs=8) as fnp,
            tc.tile_pool(name="fb", bufs=24) as fbp,
            tc.tile_pool(name="f8", bufs=2) as f8p,
            tc.tile_pool(name="mm", bufs=3, space="PSUM") as psum,
            tc.tile_pool(name="sc", bufs=3, space="PSUM") as spsum,
            tc.tile_pool(name="tp", bufs=2, space="PSUM") as tpsum,
            tc.tile_pool(name="tb", bufs=6) as tp,
            tc.tile_pool(name="al", bufs=2) as alp,
            tc.tile_pool(name="ms", bufs=1) as ms,
        ):
            # ---------------- prep: constants & weights ----------------
            eye = const.tile([P, P], f32, tag="eye")
            nc.sync.dma_start(eye[:], eye_dram[:, :])
            eye_bf = const.tile([P, P], bf16, tag="eye_bf")
            nc.vector.tensor_copy(eye_bf[:], eye[:])

            # small prep loads first so they never queue behind feature loads
            h_nat = alp.tile([X, DEC], f32, tag="esb", name="h_nat")
            nc.sync.dma_start(h_nat[:], hid[:, :])
            b1_nat = alp.tile([1, ATT], f32, tag="scores", name="b1_nat")
            nc.sync.dma_start(b1_nat[:], w1b[None, :])
            b2_nat = alp.tile([1, ATT], f32, tag="esb", name="b2_nat")
            nc.sync.dma_start(b2_nat[:], w2b[None, :])
            v_nat = alp.tile([1, ATT], f32, tag="scores", name="v_nat")
            nc.sync.dma_start(v_nat[:], vw[None, :])

            # W1 -> fp8 x64 in DoubleRow pair layout: w1d[q][p, i, m] =
            # 64*W1[(2q+i)*128 + p, m]
            w1d = []
            for q in range(NE // 2):
                t = const.tile([P, 2, ATT], fp8, tag=f"w1d_{q}")
                w1d.append(t)
            for e in range(NE):
                stage = fnp.tile([P, ATT], f32, tag="fn", name=f"w1s{e}")
                nc.sync.dma_start(stage[:], w1[P * e : P * (e + 1), :])
                q, i = divmod(e, 2)
                nc.vector.tensor_scalar_mul(w1d[q][:, i, :], stage[:], WSCALE)

            w2t = []
            for e in range(ND):
                t = const.tile([P, ATT], bf16, tag=f"w2_{e}")
                nc.gpsimd.dma_start(t[:], w2[P * e : P * (e + 1), :])
                w2t.append(t)

            # ---------------- per-example staging helpers ----------------
            fnat_map = {}   # (x, c) -> f32 natural tile
            fb_map = {}     # x -> {c: bf16 natural tile}
            f8_map = {}     # x -> ft8 tile  [P, NE*L] fp8 (features x32)

            def emit_dma_half(x, half):
                for c in range(4 * half, 4 * half + 4):
                    fnat = fnp.tile([P, ENC], f32, tag="fn", name=f"fn{x}_{c}")
                    fnat_map[(x, c)] = fnat
                    for q2 in range(4):
                        nc.sync.dma_start(
                            fnat[:, 256 * q2 : 256 * (q2 + 1)],
                            feats[x, P * c : P * (c + 1), 256 * q2 : 256 * (q2 + 1)],
                        )

            def emit_cast(x, k):
                # k = 2*c + hh : cast half hh of chunk c, f32 -> bf16 on DVE
                c, hh = divmod(k, 2)
                if hh == 0:
                    fb_map.setdefault(x, {})[c] = fbp.tile([P, ENC], bf16, tag="fb", name=f"fb{x}_{c}")
                fb = fb_map[x][c]
                nc.vector.tensor_copy(
                    fb[:, LH * hh : LH * (hh + 1)],
                    fnat_map[(x, c)][:, LH * hh : LH * (hh + 1)],
                )
                if hh == 1:
                    fnat_map.pop((x, c))

            def ft_view(ft):
                return ft.rearrange("p (e lc c) -> p e lc c", e=NE, lc=NE)

            def emit_transpose_group(x, g):
                # g = 2*lc + h : transpose blocks (e in [4h, 4h+4), l-chunk lc),
                # then one DVE copy PSUM bf16 -> ft8 fp8 with the x32 scale
                lc, h = divmod(g, 2)
                if g == 0:
                    f8_map[x] = f8p.tile([P, NE * L], fp8, tag="f8", name=f"f8{x}")
                f8 = f8_map[x]
                fb = fb_map[x][lc]
                tps = tpsum.tile([P, 4 * P], bf16, tag="tp")
                for j in range(4):
                    e = 4 * h + j
                    nc.tensor.transpose(
                        tps[:, P * j : P * (j + 1)],
                        fb[:, P * e : P * (e + 1)],
                        eye_bf[:],
                    )
                dst = ft_view(f8)[:, 4 * h : 4 * h + 4, lc, :]
                src = tps.rearrange("p (e c) -> p e c", e=4)
                nc.vector.tensor_scalar_mul(dst, src, FSCALE)

            # ---------------- prologue ----------------
            emit_dma_half(0, 0)
            emit_dma_half(0, 1)
            for k in range(16):
                emit_cast(0, k)
            emit_dma_half(1, 0)
            emit_dma_half(1, 1)

            # hT_all[p, c, x] = hid[x, 128c + p] via natural load + PE transpose
            hn_bf = ms.tile([X, DEC], bf16, tag="hn_bf")
            nc.vector.tensor_copy(hn_bf[:], h_nat[:])
            hTb = ms.tile([P, ND, X], bf16, tag="hTb")
            for c in range(ND):
                tps_h = tpsum.tile([P, X], bf16, tag="tp", name=f"tpsh{c}")
                nc.tensor.transpose(tps_h[:], hn_bf[:, P * c : P * (c + 1)], eye_bf[0:X, 0:X])
                nc.vector.tensor_copy(hTb[:, c, :], tps_h[:])

            # bias / V vectors transposed into [128, NA] (partition = within-chunk)
            def load_transposed_vec(nat, name):
                tps_v = tpsum.tile([P, NA], f32, tag="tp", name=f"tps_{name}")
                for c in range(NA):
                    nc.tensor.transpose(
                        tps_v[:, c : c + 1], nat[:, P * c : P * (c + 1)], eye[0:1, 0:1]
                    )
                dst = ms.tile([P, NA], f32, tag=name, name=name)
                nc.vector.tensor_copy(dst[:], tps_v[:])
                return dst

            b1T = load_transposed_vec(b1_nat, "b1T")
            b2T = load_transposed_vec(b2_nat, "b2T")
            vT = load_transposed_vec(v_nat, "vT")
            bT = ms.tile([P, NA], f32, tag="bT")
            nc.vector.tensor_add(bT[:], b1T[:], b2T[:])

            # V in fp8 x64 DoubleRow pair layout: vwd[p, i, j] = 64*V[(2j+i)*128+p]
            # (free-dim padded to 16 so the Ko step is 16 B)
            vwd = ms.tile([P, 2, 16], fp8, tag="vwd")
            nc.vector.tensor_scalar_mul(
                vwd[:, :, 0:4], vT.rearrange("p (j two) -> p two j", two=2), WSCALE
            )

            # proj_h transposed, plus bias: phb[p, a, x]
            phb = ms.tile([P, NA, X], f32, tag="phb")
            for a in range(NA):
                ph_ps = psum.tile([P, X], f32, tag="mm")
                for e in range(ND):
                    nc.tensor.matmul(
                        ph_ps[:],
                        w2t[e][:, P * a : P * (a + 1)],
                        hTb[:, e, :],
                        start=(e == 0),
                        stop=(e == ND - 1),
                    )
                nc.vector.tensor_scalar_add(phb[:, a, :], ph_ps[:], bT[:, a : a + 1])

            # stage example 0 fully (transposes write ft8), example 1 casts
            for g in range(16):
                emit_transpose_group(0, g)
            for k in range(16):
                emit_cast(1, k)

            # ---------------- main per-example pipeline ----------------
            pending = []

            def flush_pending():
                for sc_ap, j, tb_ap in pending:
                    nc.tensor.matmul(
                        sc_ap,
                        vwd[:, :, j : j + 1],
                        tb_ap,
                        start=(j == 0),
                        stop=(j == 3),
                        perf_mode=DR,
                    )
                pending.clear()

            pending_ctx = []

            def flush_ctx(n):
                for _ in range(min(n, len(pending_ctx))):
                    pending_ctx.pop(0)()

            for x in range(X):
                cast_for = x + 2 if x + 2 < X else None
                trans_for = x + 1 if x + 1 < X else None
                if cast_for is not None:
                    emit_dma_half(cast_for, 0)

                ft8v = f8_map[x].rearrange("p (e l) -> p e l", e=NE)
                sc_h = {}
                tb3 = None
                for b in range(16):
                    lh, a = divmod(b, 8)
                    j, i = divmod(a, 2)
                    if a == 0:
                        sc_h[lh] = spsum.tile([1, LH], f32, tag="sc", name=f"sch{x}_{lh}")
                    if i == 0:
                        tb3 = tp.tile([P, 2, LH], fp8, tag="tb")
                    pp = psum.tile([P, LH], f32, tag="mm")
                    for q in range(4):
                        nc.tensor.matmul(
                            pp[:],
                            w1d[q][:, :, P * a : P * (a + 1)],
                            ft8v[:, 2 * q : 2 * q + 2, LH * lh : LH * (lh + 1)],
                            start=(q == 0),
                            stop=(q == 3),
                            perf_mode=DR,
                        )
                        if q == 1:
                            flush_pending()

                    # deferred context matmuls of example x-1 (2 per block)
                    if b < 8:
                        flush_ctx(2)
                    # scheduled pipeline work for later examples
                    if cast_for is not None:
                        if b == 4:
                            emit_dma_half(cast_for, 1)
                        if 5 <= b < 9:
                            emit_cast(cast_for, 2 * (b - 5))
                            emit_cast(cast_for, 2 * (b - 5) + 1)
                        elif 12 <= b:
                            emit_cast(cast_for, 2 * (b - 12) + 8)
                            emit_cast(cast_for, 2 * (b - 12) + 9)
                    if trans_for is not None and b >= 8:
                        emit_transpose_group(trans_for, 2 * (b - 8))
                        emit_transpose_group(trans_for, 2 * (b - 8) + 1)

                    nc.scalar.activation(
                        tb3[:, i, :], pp[:], Tanh,
                        bias=phb[:, a, x : x + 1], scale=PSCALE,
                    )
                    if i == 1:
                        pending.append((sc_h[lh][:], j, tb3[:]))

                flush_pending()

                # softmax over L on partition 0 (scores carry a x64 scale)
                scores = alp.tile([1, L], f32, tag="scores")
                nc.vector.tensor_copy(scores[:, 0:LH], sc_h[0][:])
                nc.vector.tensor_copy(scores[:, LH:L], sc_h[1][:])
                negm = alp.tile([1, 1], f32, tag="negm")
                nc.vector.tensor_reduce(
                    negm[:], scores[:], axis=AX, op=mybir.AluOpType.max, negate=True
                )
                negm_s = alp.tile([1, 1], f32, tag="negm_s")
                nc.vector.tensor_scalar_mul(negm_s[:], negm[:], SSCALE)
                esb = alp.tile([1, L], f32, tag="esb")
                ssum = alp.tile([1, 1], f32, tag="ssum")
                nc.scalar.activation(
                    esb[:], scores[:], Exp, bias=negm_s[:], scale=SSCALE,
                    accum_out=ssum[:],
                )
                rinv = alp.tile([1, 1], f32, tag="rinv")
                nc.vector.reciprocal(rinv[:], ssum[:])
                a32 = alp.tile([1, L], f32, tag="scores", name=f"a32_{x}")
                nc.vector.tensor_scalar_mul(a32[:], esb[:], rinv[:])
                nc.sync.dma_start(alpha_o[x, :], a32[:])

                # context on PE against the natural bf16 tiles, deferred into
                # x+1's early blocks so the PE never waits on the softmax
                def make_ctx(x, a32):
                    fb_x = fb_map[x]
                    alT = alp.tile([P, NE], bf16, tag="alT", name=f"alT{x}")
                    ctr2 = alp.tile([1, ENC], f32, tag="esb", name=f"ctr2_{x}")
                    state = {}

                    def stage0():
                        tps_a = tpsum.tile([P, NE], f32, tag="tp", name=f"tpsa{x}")
                        for lc in range(NE):
                            nc.tensor.transpose(
                                tps_a[:, lc : lc + 1], a32[:, P * lc : P * (lc + 1)],
                                eye[0:1, 0:1],
                            )
                        nc.vector.tensor_copy(alT[:], tps_a[:])

                    def half(eh):
                        def run():
                            cps = psum.tile([1, LH], f32, tag="mm", name=f"cps{x}_{eh}")
                            state[eh] = cps
                            for lc in range(NE):
                                nc.tensor.matmul(
                                    cps[:],
                                    alT[:, lc : lc + 1],
                                    fb_x[lc][:, LH * eh : LH * (eh + 1)],
                                    start=(lc == 0),
                                    stop=(lc == NE - 1),
                                )
                            nc.vector.tensor_copy(ctr2[:, LH * eh : LH * (eh + 1)], state[eh][:])
                            if eh == 1:
                                nc.sync.dma_start(ctx_o[x, :], ctr2[:])
                        return run

                    return [stage0, half(0), half(1)]

                pending_ctx.extend(make_ctx(x, a32))
                if x == X - 1:
                    flush_ctx(len(pending_ctx))

    nc.compile()
    return nc


def kernel(features, hidden_state, W1_w, W1_b, W2_w, W2_b, V_w, V_b):
    from concourse.bass_utils import run_bass_kernel_spmd

    if "nc" not in _CACHE:
        _CACHE["nc"] = _build()
    nc = _CACHE["nc"]

    features = np.ascontiguousarray(np.asarray(features, dtype=np.float32))
    hidden_state = np.ascontiguousarray(np.asarray(hidden_state, dtype=np.float32))
    W1_w = np.ascontiguousarray(np.asarray(W1_w, dtype=np.float32))
    W1_b = np.ascontiguousarray(np.asarray(W1_b, dtype=np.float32))
    W2_w = np.ascontiguousarray(np.asarray(W2_w, dtype=np.float32))
    W2_b = np.ascontiguousarray(np.asarray(W2_b, dtype=np.float32))
    V_w = np.ascontiguousarray(np.asarray(V_w, dtype=np.float32))

    in_maps = []
    for c in range(N_CORES):
        in_maps.append(
            {
                "features": np.ascontiguousarray(features[c * X : (c + 1) * X]),
                "hidden_state": np.ascontiguousarray(hidden_state[c * X : (c + 1) * X]),
                "W1_w": W1_w,
                "W1_b": W1_b,
                "W2_w": W2_w,
                "W2_b": W2_b,
                "V_w": V_w,
            }
        )

    res = run_bass_kernel_spmd(nc, in_maps, list(range(N_CORES)), **_CACHE.get("run_kwargs", {}))
    _CACHE["last_result"] = res
    alpha = np.concatenate([res.results[c]["alpha"] for c in range(N_CORES)], axis=0)
    context = np.concatenate([res.results[c]["context"] for c in range(N_CORES)], axis=0)
    return alpha, context


# revision 10
# speedup vs baseline: 1.1671x; 1.0505x over previous
"""Trainium2 Bass kernel for additive (Bahdanau) attention — fp8 DoubleRow.

reference:
    proj_f = features @ W1_w + W1_b          # [B, L, ATT]
    proj_h = (hidden @ W2_w + W2_b)[:, None] # [B, 1, ATT]
    scores = tanh(proj_f + proj_h) @ V_w + V_b   # [B, L]
    alpha  = softmax(scores, axis=1)
    context = einsum('bl,ble->be', alpha, features)
    returns (alpha, context)

Sharding: data-parallel over batch B=64 across 8 cores (8 examples/core).
Weights replicated. No collectives.

Per-core algorithm (X = 8 examples), evolved from the bf16 baseline:
  - main GEMM and V-dot run in fp8e4 with perf_mode=DoubleRow (2 fp8
    weights/PE cell, K=256 per matmul).  To dodge e4m3 subnormals,
    features are scaled x32 and W1/V x64; the tanh activation descales
    via its scale arg (2^-11) and exp via scale 1/64 (softmax is
    shift-invariant so V_b is dropped, and the max-subtract works on the
    x64 scores with a rescaled bias).
  - context needs bf16 features for accuracy (fp8 features give ~3e-2
    rel err), so TWO transposed copies are kept: ft16 (bf16, feeds the
    DVE context STT) and ft8 = fp8(32 * ft16) via one strided DVE cast.
  - f32->bf16 feature casts moved from DVE to the Scalar engine (ACT),
    doled out two per GEMM block so they never head-of-line-block the
    tanh ops behind slow DMA arrivals.
  - software pipeline per example x's 16 GEMM blocks:
      blocks 0-3   : (DMA for x+2 chunks 0-3 issued at loop top)
      blocks 4-7   : ACT casts for x+2 chunks 0-3
      block  8     : DMA issue for x+2 chunks 4-7
      blocks 8-15  : PE transposes for x+1 (2 groups/block) + ft8 casts
      blocks 12-15 : ACT casts for x+2 chunks 4-7
  - scores accumulate in PSUM [1, 512] via DoubleRow V-dot matmuls that
    trail the tanh by one block so the PE never waits on ACT.
  - context on DVE: scalar_tensor_tensor over ft16 with alpha replicated
    across partitions (gpsimd partition_broadcast); bf16 product scratch.
    Last example: context on PE against the natural bf16 tiles (shorter
    serial tail).
"""

import numpy as np

B, L, ENC, DEC, ATT = 64, 1024, 1024, 1024, 1024
N_CORES = 8
X = B // N_CORES  # examples per core
P = 128
NE = ENC // P  # 8
NA = ATT // P  # 8
ND = DEC // P  # 8
LH = 512       # free-dim half for fp32 PSUM bank
NL = L // LH   # 2

FSCALE = 32.0                      # feature scale into fp8
WSCALE = 64.0                      # W1 / V scale into fp8
PSCALE = 1.0 / (FSCALE * WSCALE)   # descale inside tanh
SSCALE = 1.0 / WSCALE              # descale inside exp

_CACHE = {}


def _build():
    import concourse.bacc as bacc
    import concourse.mybir as mybir
    import concourse.tile as tile

    f32, bf16 = mybir.dt.float32, mybir.dt.bfloat16
    fp8 = mybir.dt.float8e4
    Tanh = mybir.ActivationFunctionType.Tanh
    Exp = mybir.ActivationFunctionType.Exp
    DR = mybir.MatmulPerfMode.DoubleRow
    mult = mybir.AluOpType.mult
    AX = mybir.AxisListType.X

    nc = bacc.Bacc("TRN2", target_bir_lowering=False, debug=False, num_devices=N_CORES)

    feats = nc.declare_dram_parameter("features", [X, L, ENC], f32, isOutput=False)
    hid = nc.declare_dram_parameter("hidden_state", [X, DEC], f32, isOutput=False)
    w1 = nc.declare_dram_parameter("W1_w", [ENC, ATT], f32, isOutput=False)
    w1b = nc.declare_dram_parameter("W1_b", [ATT], f32, isOutput=False)
    w2 = nc.declare_dram_parameter("W2_w", [DEC, ATT], f32, isOutput=False)
    w2b = nc.declare_dram_parameter("W2_b", [ATT], f32, isOutput=False)
    vw = nc.declare_dram_parameter("V_w", [ATT], f32, isOutput=False)
    alpha_o = nc.declare_dram_parameter("alpha", [X, L], f32, isOutput=True)
    ctx_o = nc.declare_dram_parameter("context", [X, ENC], f32, isOutput=True)

    eye_dram = nc.inline_tensor(np.eye(P, dtype=np.float32), "eye128")

    with tile.TileContext(nc) as tc:
        with (
            tc.tile_pool(name="const", bufs=1) as const,
            tc.tile_pool(name="fn", bufs=16) as fnp,
            tc.tile_pool(name="fb", bufs=24) as fbp,
            tc.tile_pool(name="f8", bufs=2) as f8p,
            tc.tile_pool(name="mm", bufs=3, space="PSUM") as psum,
            tc.tile_pool(name="sc", bufs=3, space="PSUM") as spsum,
            tc.tile_pool(name="tp", bufs=2, space="PSUM") as tpsum,
            tc.tile_pool(name="tb", bufs=6) as tp,
            tc.tile_pool(name="al", bufs=2) as alp,
            tc.tile_pool(name="ms", bufs=1) as ms,
        ):
            # ---------------- prep: constants & weights ----------------
            eye = const.tile([P, P], f32, tag="eye")
            nc.sync.dma_start(eye[:], eye_dram[:, :])
            eye_bf = const.tile([P, P], bf16, tag="eye_bf")
            nc.vector.tensor_copy(eye_bf[:], eye[:])

            # small prep loads first so they never queue behind feature loads
            h_nat = alp.tile([X, DEC], f32, tag="esb", name="h_nat")
            nc.sync.dma_start(h_nat[:], hid[:, :])
            b1_nat = alp.tile([1, ATT], f32, tag="scores", name="b1_nat")
            nc.sync.dma_start(b1_nat[:], w1b[None, :])
            b2_nat = alp.tile([1, ATT], f32, tag="esb", name="b2_nat")
            nc.sync.dma_start(b2_nat[:], w2b[None, :])
            v_nat = alp.tile([1, ATT], f32, tag="scores", name="v_nat")
            nc.sync.dma_start(v_nat[:], vw[None, :])

            # W1 -> fp8 x64 in DoubleRow pair layout: w1d[q][p, i, m] =
            # 64*W1[(2q+i)*128 + p, m]
            w1d = []
            for q in range(NE // 2):
                t = const.tile([P, 2, ATT], fp8, tag=f"w1d_{q}")
                w1d.append(t)
            for e in range(NE):
                stage = fnp.tile([P, ATT], f32, tag="fn", name=f"w1s{e}")
                nc.sync.dma_start(stage[:], w1[P * e : P * (e + 1), :])
                q, i = divmod(e, 2)
                nc.vector.tensor_scalar_mul(w1d[q][:, i, :], stage[:], WSCALE)

            w2t = []
            for e in range(ND):
                t = const.tile([P, ATT], bf16, tag=f"w2_{e}")
                nc.gpsimd.dma_start(t[:], w2[P * e : P * (e + 1), :])
                w2t.append(t)

            # ---------------- per-example staging helpers ----------------
            fnat_map = {}   # (x, c) -> f32 natural tile
            fb_map = {}     # x -> {c: bf16 natural tile}
            f8_map = {}     # x -> ft8 tile  [P, NE*L] fp8 (features x32)

            def emit_dma_half(x, half):
                for c in range(4 * half, 4 * half + 4):
                    fnat = fnp.tile([P, ENC], f32, tag="fn", name=f"fn{x}_{c}")
                    fnat_map[(x, c)] = fnat
                    for q2 in range(4):
                        nc.sync.dma_start(
                            fnat[:, 256 * q2 : 256 * (q2 + 1)],
                            feats[x, P * c : P * (c + 1), 256 * q2 : 256 * (q2 + 1)],
                        )

            def emit_cast(x, k):
                # k = 2*c + hh : cast half hh of chunk c, f32 -> bf16 on DVE
                c, hh = divmod(k, 2)
                if hh == 0:
                    fb_map.setdefault(x, {})[c] = fbp.tile([P, ENC], bf16, tag="fb", name=f"fb{x}_{c}")
                fb = fb_map[x][c]
                nc.vector.tensor_copy(
                    fb[:, LH * hh : LH * (hh + 1)],
                    fnat_map[(x, c)][:, LH * hh : LH * (hh + 1)],
                )
                if hh == 1:
                    fnat_map.pop((x, c))

            def ft_view(ft):
                return ft.rearrange("p (e lc c) -> p e lc c", e=NE, lc=NE)

            def emit_transpose_group(x, g):
                # g = 2*lc + h : transpose blocks (e in [4h, 4h+4), l-chunk lc),
                # then one DVE copy PSUM bf16 -> ft8 fp8 with the x32 scale
                lc, h = divmod(g, 2)
                if g == 0:
                    f8_map[x] = f8p.tile([P, NE * L], fp8, tag="f8", name=f"f8{x}")
                f8 = f8_map[x]
                fb = fb_map[x][lc]
                tps = tpsum.tile([P, 4 * P], bf16, tag="tp")
                for j in range(4):
                    e = 4 * h + j
                    nc.tensor.transpose(
                        tps[:, P * j : P * (j + 1)],
                        fb[:, P * e : P * (e + 1)],
                        eye_bf[:],
                    )
                dst = ft_view(f8)[:, 4 * h : 4 * h + 4, lc, :]
                src = tps.rearrange("p (e c) -> p e c", e=4)
                nc.vector.tensor_scalar_mul(dst, src, FSCALE)

            # ---------------- prologue ----------------
            emit_dma_half(0, 0)
            emit_dma_half(0, 1)
            for k in range(16):
                emit_cast(0, k)
            emit_dma_half(1, 0)
            emit_dma_half(1, 1)

            # hT_all[p, c, x] = hid[x, 128c + p] via natural load + PE transpose
            hn_bf = ms.tile([X, DEC], bf16, tag="hn_bf")
            nc.vector.tensor_copy(hn_bf[:], h_nat[:])
            hTb = ms.tile([P, ND, X], bf16, tag="hTb")
            for c in range(ND):
                tps_h = tpsum.tile([P, X], bf16, tag="tp", name=f"tpsh{c}")
                nc.tensor.transpose(tps_h[:], hn_bf[:, P * c : P * (c + 1)], eye_bf[0:X, 0:X])
                nc.vector.tensor_copy(hTb[:, c, :], tps_h[:])

            # bias / V vectors transposed into [128, NA] (partition = within-chunk)
            def load_transposed_vec(nat, name):
                tps_v = tpsum.tile([P, NA], f32, tag="tp", name=f"tps_{name}")
                for c in range(NA):
                    nc.tensor.transpose(
                        tps_v[:, c : c + 1], nat[:, P * c : P * (c + 1)], eye[0:1, 0:1]
                    )
                dst = ms.tile([P, NA], f32, tag=name, name=name)
                nc.vector.tensor_copy(dst[:], tps_v[:])
                return dst

            b1T = load_transposed_vec(b1_nat, "b1T")
            b2T = load_transposed_vec(b2_nat, "b2T")
            vT = load_transposed_vec(v_nat, "vT")
            bT = ms.tile([P, NA], f32, tag="bT")
            nc.vector.tensor_add(bT[:], b1T[:], b2T[:])

            # V in fp8 x64 DoubleRow pair layout: vwd[p, i, j] = 64*V[(2j+i)*128+p]
            # (free-dim padded to 16 so the Ko step is 16 B)
            vwd = ms.tile([P, 2, 16], fp8, tag="vwd")
            nc.vector.tensor_scalar_mul(
                vwd[:, :, 0:4], vT.rearrange("p (j two) -> p two j", two=2), WSCALE
            )

            # proj_h transposed, plus bias: phb[p, a, x]
            phb = ms.tile([P, NA, X], f32, tag="phb")
            for a in range(NA):
                ph_ps = psum.tile([P, X], f32, tag="mm")
                for e in range(ND):
                    nc.tensor.matmul(
                        ph_ps[:],
                        w2t[e][:, P * a : P * (a + 1)],
                        hTb[:, e, :],
                        start=(e == 0),
                        stop=(e == ND - 1),
                    )
                nc.vector.tensor_scalar_add(phb[:, a, :], ph_ps[:], bT[:, a : a + 1])

            # stage example 0 fully (transposes write ft8), example 1 casts,
            # example 2 feature prefetch
            for g in range(16):
                emit_transpose_group(0, g)
            emit_dma_half(2, 0)
            emit_dma_half(2, 1)
            for k in range(16):
                emit_cast(1, k)

            # ---------------- main per-example pipeline ----------------
            pending = []

            def flush_pending(cur_b=10**6):
                keep = []
                for sc_ap, j, tb_ap, b_emit in pending:
                    if b_emit <= cur_b - 2:
                        nc.tensor.matmul(
                            sc_ap,
                            vwd[:, :, j : j + 1],
                            tb_ap,
                            start=(j == 0),
                            stop=(j == 3),
                            perf_mode=DR,
                        )
                    else:
                        keep.append((sc_ap, j, tb_ap, b_emit))
                pending[:] = keep

            pending_ctx = []

            def flush_ctx(n):
                for _ in range(min(n, len(pending_ctx))):
                    pending_ctx.pop(0)()

            for x in range(X):
                cast_for = x + 2 if x + 2 < X else None
                trans_for = x + 1 if x + 1 < X else None
                dma_for = x + 3 if x + 3 < X else None
                if dma_for is not None:
                    emit_dma_half(dma_for, 0)

                ft8v = f8_map[x].rearrange("p (e l) -> p e l", e=NE)
                sc_h = {}
                tb3 = None
                for b in range(16):
                    lh, a = divmod(b, 8)
                    j, i = divmod(a, 2)
                    if a == 0:
                        sc_h[lh] = spsum.tile([1, LH], f32, tag="sc", name=f"sch{x}_{lh}")
                    if i == 0:
                        tb3 = tp.tile([P, 2, LH], fp8, tag="tb")
                    pp = psum.tile([P, LH], f32, tag="mm")
                    for q in range(4):
                        nc.tensor.matmul(
                            pp[:],
                            w1d[q][:, :, P * a : P * (a + 1)],
                            ft8v[:, 2 * q : 2 * q + 2, LH * lh : LH * (lh + 1)],
                            start=(q == 0),
                            stop=(q == 3),
                            perf_mode=DR,
                        )
                        if q == 1:
                            flush_pending(b)

                    # deferred context matmuls of example x-1, behind the
                    # softmax latency
                    if 4 <= b < 8:
                        flush_ctx(1)
                    # scheduled pipeline work for later examples
                    if dma_for is not None and b == 8:
                        emit_dma_half(dma_for, 1)
                    if cast_for is not None:
                        emit_cast(cast_for, b)
                    if trans_for is not None and b >= 8:
                        emit_transpose_group(trans_for, 2 * (b - 8))
                        emit_transpose_group(trans_for, 2 * (b - 8) + 1)

                    nc.scalar.activation(
                        tb3[:, i, :], pp[:], Tanh,
                        bias=phb[:, a, x : x + 1], scale=PSCALE,
                    )
                    if i == 1:
                        pending.append((sc_h[lh][:], j, tb3[:], b))

                flush_pending()

                # softmax over L on partition 0 (scores carry a x64 scale)
                scores = alp.tile([1, L], f32, tag="scores")
                nc.vector.tensor_copy(scores[:, 0:LH], sc_h[0][:])
                nc.vector.tensor_copy(scores[:, LH:L], sc_h[1][:])
                negm = alp.tile([1, 1], f32, tag="negm")
                nc.vector.tensor_reduce(
                    negm[:], scores[:], axis=AX, op=mybir.AluOpType.max, negate=True
                )
                negm_s = alp.tile([1, 1], f32, tag="negm_s")
                nc.vector.tensor_scalar_mul(negm_s[:], negm[:], SSCALE)
                esb = alp.tile([1, L], f32, tag="esb")
                ssum = alp.tile([1, 1], f32, tag="ssum")
                nc.scalar.activation(
                    esb[:], scores[:], Exp, bias=negm_s[:], scale=SSCALE,
                    accum_out=ssum[:],
                )
                rinv = alp.tile([1, 1], f32, tag="rinv")
                nc.vector.reciprocal(rinv[:], ssum[:])
                a32 = alp.tile([1, L], f32, tag="scores", name=f"a32_{x}")
                nc.vector.tensor_scalar_mul(a32[:], esb[:], rinv[:])
                nc.sync.dma_start(alpha_o[x, :], a32[:])

                # context on PE against the natural bf16 tiles, deferred into
                # x+1's early blocks so the PE never waits on the softmax
                def make_ctx(x, a32):
                    fb_x = fb_map[x]
                    alT = alp.tile([P, NE], bf16, tag="alT", name=f"alT{x}")
                    ctr2 = alp.tile([1, ENC], f32, tag="esb", name=f"ctr2_{x}")
                    state = {}

                    def stage0():
                        tps_a = tpsum.tile([P, NE], f32, tag="tp", name=f"tpsa{x}")
                        for lc in range(NE):
                            nc.tensor.transpose(
                                tps_a[:, lc : lc + 1], a32[:, P * lc : P * (lc + 1)],
                                eye[0:1, 0:1],
                            )
                        nc.vector.tensor_copy(alT[:], tps_a[:])

                    def half(eh):
                        def run():
                            cps = psum.tile([1, LH], f32, tag="mm", name=f"cps{x}_{eh}")
                            state[eh] = cps
                            for lc in range(NE):
                                nc.tensor.matmul(
                                    cps[:],
                                    alT[:, lc : lc + 1],
                                    fb_x[lc][:, LH * eh : LH * (eh + 1)],
                                    start=(lc == 0),
                                    stop=(lc == NE - 1),
                                )
                            nc.vector.tensor_copy(ctr2[:, LH * eh : LH * (eh + 1)], state[eh][:])
                            if eh == 1:
                                nc.sync.dma_start(ctx_o[x, :], ctr2[:])
                        return run

                    return [stage0, half(0), half(1)]

                pending_ctx.extend(make_ctx(x, a32))
                if x == X - 1:
                    flush_ctx(len(pending_ctx))

    nc.compile()
    return nc


def kernel(features, hidden_state, W1_w, W1_b, W2_w, W2_b, V_w, V_b):
    from concourse.bass_utils import run_bass_kernel_spmd

    if "nc" not in _CACHE:
        _CACHE["nc"] = _build()
    nc = _CACHE["nc"]

    features = np.ascontiguousarray(np.asarray(features, dtype=np.float32))
    hidden_state = np.ascontiguousarray(np.asarray(hidden_state, dtype=np.float32))
    W1_w = np.ascontiguousarray(np.asarray(W1_w, dtype=np.float32))
    W1_b = np.ascontiguousarray(np.asarray(W1_b, dtype=np.float32))
    W2_w = np.ascontiguousarray(np.asarray(W2_w, dtype=np.float32))
    W2_b = np.ascontiguousarray(np.asarray(W2_b, dtype=np.float32))
    V_w = np.ascontiguousarray(np.asarray(V_w, dtype=np.float32))

    in_maps = []
    for c in range(N_CORES):
        in_maps.append(
            {
                "features": np.ascontiguousarray(features[c * X : (c + 1) * X]),
                "hidden_state": np.ascontiguousarray(hidden_state[c * X : (c + 1) * X]),
                "W1_w": W1_w,
                "W1_b": W1_b,
                "W2_w": W2_w,
                "W2_b": W2_b,
                "V_w": V_w,
            }
        )

    res = run_bass_kernel_spmd(nc, in_maps, list(range(N_CORES)), **_CACHE.get("run_kwargs", {}))
    _CACHE["last_result"] = res
    alpha = np.concatenate([res.results[c]["alpha"] for c in range(N_CORES)], axis=0)
    context = np.concatenate([res.results[c]["context"] for c in range(N_CORES)], axis=0)
    return alpha, context
